# revision 3
# baseline (speedup 1.0000x reference)
"""BiLSTM-CRF Trainium2 kernel (self-contained).

Strategy
--------
Data-parallel over batch: B=32 sequences -> 8 cores x 4 sequences.
Per core, each LSTM direction's recurrence is broken into 32 chunks of 64
steps per sequence (128 independent chains = 4 seqs x 32 chunks), each chunk
preceded by W=16 warm-up steps.  LSTM forget gates make the influence of the
warm-up start state decay like ~e^-1.6/step, so W=16 reproduces the exact
recurrence to ~1e-7.

Host->device traffic is the dominant cost (axon-tunneled cores, ~170MB/s
aggregate), so the input payload is minimized:
 - All replicated weights live in one [128, 6152] bf16 blob; each core
   receives a distinct [16, 6152] shard and the full blob is reconstructed
   on device with a NeuronLink AllGather (8x less host traffic).
 - Token ids / tags ship as raw uint8 and the per-step one-hot gather
   patterns are strided SBUF views of an on-device replicated copy
   (the chunk gather is affine: pos = c*64 + tau fwd, c*64 + 95 - tau bwd).

Layer-0 input projections are a pure function of token id, folded into a
256-entry gate table applied by one-hot matmuls.  CRF partition function =
log-semiring matrix-product tree (fully parallel).
Layout per direction: hidden on partitions [128], chains on free dim [128].
Gate order is permuted to (i, f, o, g) so sigmoid covers one contiguous span.
"""

import os
from contextlib import ExitStack

import numpy as np

# problem constants (hardcoded per contest rules)
B, S = 32, 2048
VOCAB = 256
EMB = 128
HID = 128
CHAR_EMB = 32
CHAR_HID = 32
NT = 3  # tags

NCORES = 8
BL = B // NCORES          # 4 sequences per core
C = 32                    # chunks per sequence
LC = S // C               # 64 chunk length
W = 16                    # warm-up steps
L = LC + W                # 80 local steps per phase
NCH = BL * C              # 128 chains per direction
PAD = W                   # h-buffer padding columns each side
SCR = PAD + BL * S + PAD  # scratch col offset = 8224
HB_W = SCR + 128          # h buffer width = 8352
TOK = BL * S              # 8192 tokens per core
NQ = 64                   # tokens per partition in wide layout (t = p*64 + q)

# weight blob column offsets (all bf16, gate cols i,f,o,g; g pre-scaled x2)
WOFF = {
    "whh0_f": 0, "whh0_b": 512, "whh1_f": 1024, "whh1_b": 1536,
    "wih1_ff": 2048, "wih1_fb": 2560, "wih1_bf": 3072, "wih1_bb": 3584,
    "tab_lo_f": 4096, "tab_hi_f": 4608, "tab_lo_b": 5120, "tab_hi_b": 5632,
    "wtag_f": 6144, "wtag_b": 6147,
}
NBLOB = 6152
SHR = 128 // NCORES       # blob rows per core shard = 16

# st8 flat layout (uint8): seq_pad [4,2080] | tags_wide [128,64] | prev [128,64]
SEQ_PAD = 2080            # 16 zeros | 2048 tokens | 16 zeros
ST_SEQ = 0
ST_TGW = BL * SEQ_PAD     # 8320
ST_TGP = ST_TGW + 128 * NQ  # 16512
ST_TOT = ST_TGP + 128 * NQ  # 24704 = 4 * 6176
ST_COLS = ST_TOT // BL

# smalls [4, 304] f32 flat layout
SM_COLS = 304
SM_START3 = 0   # [r, 0:3] start_trans
SM_END3 = 3     # [r, 3:6] end_trans
SM_OH0 = 6      # [r, 6:9]
SM_OHL = 9      # [r, 9:12]
SM_TRANS = 12   # [0, 12:21]
SM_STARTR = 21  # [0, 21:24]
SM_BTAG = 24    # [t, 24] t=0..2
SM_B1 = 48      # [r, 48:304] = b1cat[256r:256r+256]


def _sigmoid(x):
    return 1.0 / (1.0 + np.exp(-x))


# gate-block permutation torch(i,f,g,o) -> kernel(i,f,o,g)
def _perm_rows(w):
    # w: [512, ...] gate-major rows
    return np.concatenate([w[0:128], w[128:256], w[384:512], w[256:384]], axis=0)


def _g2(w):
    # scale the g-gate block (cols 384:512 after perm) by 2: the kernel
    # computes tanh(g) as 2*sigmoid(2g) - 1 inside one fused sigmoid op.
    w = w.copy()
    w[..., 384:512] *= 2.0
    return w


def host_prep(inputs):
    """Numpy-only input massaging: one shared weight blob + per-core u8/f32."""
    import ml_dtypes
    f32 = np.float32
    seq = np.asarray(inputs["sequences"])
    tags = np.asarray(inputs["tags"])
    word_emb = np.asarray(inputs["word_emb"], f32)
    char_emb = np.asarray(inputs["char_emb"], f32)
    cWih = np.asarray(inputs["cWih"], f32)
    cb = np.asarray(inputs["cb"], f32)
    W0ih = np.asarray(inputs["lstm0_Wih"], f32)
    W0hh = np.asarray(inputs["lstm0_Whh"], f32)
    b0 = np.asarray(inputs["lstm0_b"], f32)
    W1ih = np.asarray(inputs["lstm1_Wih"], f32)
    W1hh = np.asarray(inputs["lstm1_Whh"], f32)
    b1 = np.asarray(inputs["lstm1_b"], f32)
    Wtag = np.asarray(inputs["Wtag"], f32)
    btag = np.asarray(inputs["btag"], f32)
    start_t = np.asarray(inputs["start_trans"], f32)
    end_t = np.asarray(inputs["end_trans"], f32)
    trans = np.asarray(inputs["trans"], f32)

    # --- layer-0 token table: [2, 256, 512] (gate order i,f,o,g) ---
    ce = char_emb[np.arange(VOCAB)]  # [256, 32]
    cf = []
    for d in range(2):
        g = ce @ cWih[d].T + cb[d]
        i_, f_, g_, o_ = np.split(g, 4, axis=-1)
        c_ = _sigmoid(i_) * np.tanh(g_)
        cf.append(_sigmoid(o_) * np.tanh(c_))
    x_tok = np.concatenate([word_emb, cf[0], cf[1]], axis=-1)  # [256, 192]
    tab = np.stack(
        [x_tok @ _perm_rows(W0ih[d]).T + _perm_rows(b0[d][:, None])[:, 0]
         for d in range(2)]
    ).astype(f32)  # [2, 256, 512]

    blob = np.zeros((128, NBLOB), f32)

    def put(nm, arr):
        o = WOFF[nm]
        blob[:arr.shape[0], o:o + arr.shape[1]] = arr

    put("whh0_f", _g2(_perm_rows(W0hh[0]).T))
    put("whh0_b", _g2(_perm_rows(W0hh[1]).T))
    put("whh1_f", _g2(_perm_rows(W1hh[0]).T))
    put("whh1_b", _g2(_perm_rows(W1hh[1]).T))
    put("wih1_ff", _g2(_perm_rows(W1ih[0])[:, :128].T))
    put("wih1_fb", _g2(_perm_rows(W1ih[0])[:, 128:].T))
    put("wih1_bf", _g2(_perm_rows(W1ih[1])[:, :128].T))
    put("wih1_bb", _g2(_perm_rows(W1ih[1])[:, 128:].T))
    put("tab_lo_f", _g2(tab[0, :128]))
    put("tab_hi_f", _g2(tab[0, 128:]))
    put("tab_lo_b", _g2(tab[1, :128]))
    put("tab_hi_b", _g2(tab[1, 128:]))
    put("wtag_f", Wtag[:, :128].T)
    put("wtag_b", Wtag[:, 128:].T)
    blob16 = blob.astype(ml_dtypes.bfloat16)

    b1cat = np.concatenate(
        [_perm_rows(b1[0][:, None])[:, 0], _perm_rows(b1[1][:, None])[:, 0]])
    b1cat[384:512] *= 2.0
    b1cat[896:1024] *= 2.0

    per_core = []
    for k in range(NCORES):
        sq = seq[k * BL:(k + 1) * BL]
        tg = tags[k * BL:(k + 1) * BL]
        st8 = np.zeros(ST_TOT, np.uint8)
        sp = st8[ST_SEQ:ST_TGW].reshape(BL, SEQ_PAD)
        sp[:, 16:16 + S] = sq.astype(np.uint8)
        st8[ST_TGW:ST_TGP] = tg.astype(np.uint8).reshape(-1)
        prev = np.concatenate(
            [np.full((BL, 1), 255, np.uint8), tg[:, :-1].astype(np.uint8)],
            axis=1)
        st8[ST_TGP:ST_TOT] = prev.reshape(-1)

        sm = np.zeros((BL, SM_COLS), f32)
        sm[:, SM_START3:SM_START3 + 3] = start_t
        sm[:, SM_END3:SM_END3 + 3] = end_t
        oh0 = np.zeros((BL, 3), f32)
        ohl = np.zeros((BL, 3), f32)
        oh0[np.arange(BL), tg[:, 0]] = 1.0
        ohl[np.arange(BL), tg[:, -1]] = 1.0
        sm[:, SM_OH0:SM_OH0 + 3] = oh0
        sm[:, SM_OHL:SM_OHL + 3] = ohl
        sm[0, SM_TRANS:SM_TRANS + 9] = trans.reshape(9)
        sm[0, SM_STARTR:SM_STARTR + 3] = start_t
        sm[0:3, SM_BTAG] = btag
        sm[:, SM_B1:] = b1cat.reshape(BL, 256)

        per_core.append({
            "wsh": np.ascontiguousarray(blob16[k * SHR:(k + 1) * SHR]),
            "st8": st8.reshape(BL, ST_COLS),
            "smalls": sm,
        })
    return per_core


INPUT_SPECS = [
    ("wsh", (SHR, NBLOB), "bf16"),
    ("st8", (BL, ST_COLS), "u8"),
    ("smalls", (BL, SM_COLS), "f32"),
]


def build(tc, ins, outs):
    """Emit the whole program. ins/outs: dicts name -> bass.AP (DRAM)."""
    import concourse.bass as bass
    from concourse import mybir

    nc = tc.nc
    f32 = mybir.dt.float32
    bf = mybir.dt.bfloat16
    f16 = mybir.dt.float16
    i32 = mybir.dt.int32
    u8 = mybir.dt.uint8
    AF = mybir.ActivationFunctionType
    OP = mybir.AluOpType
    AX = mybir.AxisListType

    def fap(base, extra_off, dims, part=None):
        p = part if part is not None else base.ap[0]
        return bass.AP(tensor=base.tensor, offset=base.offset + extra_off,
                       ap=[list(p)] + [list(d) for d in dims])

    n_rep = int(os.environ.get("KREPEAT", "1"))
    with ExitStack() as ctx:
        sing = ctx.enter_context(tc.tile_pool(name="sing", bufs=1))
        dram = ctx.enter_context(tc.tile_pool(name="dram", bufs=1, space="DRAM"))

        # ---- weight blob: shard -> AllGather -> SBUF ----
        in_b = dram.tile([SHR, NBLOB], bf, name="inb", tag="inb")
        blob_d = dram.tile([128, NBLOB], bf, name="blobd", tag="blobd")
        nc.gpsimd.dma_start(out=in_b[:], in_=ins["wsh"])
        nc.gpsimd.collective_compute(
            "AllGather", OP.bypass,
            replica_groups=[list(range(NCORES))],
            ins=[in_b[:].opt()], outs=[blob_d[:].opt()])
        wblob = sing.tile([128, NBLOB], bf, name="wblob", tag="wblob")
        nc.sync.dma_start(out=wblob[:], in_=blob_d[:])

        def wv(nm, ncols=512):
            o = WOFF[nm]
            return wblob[:, o:o + ncols]

        # ---- token ids: u8 -> replicated f32 copy in SBUF ----
        sq8 = sing.tile([128, BL * SEQ_PAD], u8, name="sq8", tag="sq8")
        nc.sync.dma_start(
            out=sq8[:],
            in_=fap(ins["st8"], ST_SEQ, [[1, BL * SEQ_PAD]], part=[0, 128]))
        seq_rep = sing.tile([128, BL * SEQ_PAD], f32, name="seqrep", tag="seqrep")
        nc.vector.tensor_copy(out=seq_rep[:], in_=sq8[:])

        # per-tau one-hot input views: chain (b, c) of dir d reads
        #   fwd: b*2080 + c*64 + tau      bwd: b*2080 + c*64 + 95 - tau
        def seq_view(d, tau):
            off = tau if d == 0 else 95 - tau
            return fap(seq_rep[:], off, [[SEQ_PAD, BL], [LC, C]])

        # ---- tags: u8 wide layout -> f32 ----
        tg8 = sing.tile([128, 2, NQ], u8, name="tg8", tag="tg8")
        nc.sync.dma_start(
            out=tg8[:, 0, :],
            in_=fap(ins["st8"], ST_TGW, [[1, NQ]], part=[NQ, 128]))
        nc.sync.dma_start(
            out=tg8[:, 1, :],
            in_=fap(ins["st8"], ST_TGP, [[1, NQ]], part=[NQ, 128]))
        tgw = sing.tile([128, NQ], f32, name="tgw", tag="tgw")
        tgpw = sing.tile([128, NQ], f32, name="tgpw", tag="tgpw")
        nc.vector.tensor_copy(out=tgw[:], in_=tg8[:, 0, :])
        nc.vector.tensor_copy(out=tgpw[:], in_=tg8[:, 1, :])

        # ---- persistent SBUF state ----
        h_sb = {}  # (layer, dir) -> tile [128, HB_W]
        for l in range(2):
            for d in range(2):
                h_sb[(l, d)] = sing.tile([128, HB_W], bf, name=f"h{l}{d}",
                                         tag=f"h{l}{d}")
                nc.vector.memset(h_sb[(l, d)][:, 0:PAD], 0.0)
                nc.vector.memset(h_sb[(l, d)][:, PAD + TOK:SCR], 0.0)

        def h_rw(l, d, tau):
            """AP where step tau's h of (layer l, dir d) lives."""
            hb = h_sb[(l, d)][:]
            if tau < W:
                return fap(hb, SCR, [[C, BL], [1, C]])
            t = tau - W
            base = PAD + t if d == 0 else PAD + (LC - 1) - t
            return fap(hb, base, [[S, BL], [LC, C]])

        def h_in(src_d, pat_d, tau):
            """Layer-1 input read: layer-0 h of dir src_d at the positions
            that (dir pat_d, local step tau) consumes."""
            hb = h_sb[(0, src_d)][:]
            base = (PAD + tau - W if pat_d == 0
                    else PAD + (LC - 1) + W - tau)
            return fap(hb, base, [[S, BL], [LC, C]])

        # ---- constants ----
        ones1 = sing.tile([1, 128], bf, name="ones1", tag="ones1")
        nc.vector.memset(ones1[:], 1.0)
        b1f32 = sing.tile([1, 2, 512], f32, name="b1f32", tag="b1f32")
        nc.sync.dma_start(
            out=b1f32[:],
            in_=fap(ins["smalls"], SM_B1, [[SM_COLS, BL], [1, 256]],
                    part=[0, 1]))
        b1row = sing.tile([1, 2, 512], bf, name="b1row", tag="b1row")
        nc.vector.tensor_copy(out=b1row[:], in_=b1f32[:])

        iota_i = sing.tile([128, 2], i32, name="iotai", tag="iotai")
        nc.gpsimd.iota(iota_i[:, 0:1], pattern=[[0, 1]], base=0,
                       channel_multiplier=1)
        nc.gpsimd.iota(iota_i[:, 1:2], pattern=[[0, 1]], base=128,
                       channel_multiplier=1)
        iota_f = sing.tile([128, 2], f32, name="iotaf", tag="iotaf")
        nc.vector.tensor_copy(out=iota_f[:], in_=iota_i[:])

        # chunk-boundary state-zero mask from an on-device chain iota
        it_i = sing.tile([128, NCH], i32, name="iti", tag="iti")
        nc.gpsimd.iota(it_i[:], pattern=[[0, BL], [1, C]], base=0,
                       channel_multiplier=0)
        it_f = sing.tile([128, NCH], f32, name="itf", tag="itf")
        nc.vector.tensor_copy(out=it_f[:], in_=it_i[:])
        maskz = sing.tile([128, 2, NCH], bf, name="maskz", tag="maskz")
        nc.vector.tensor_scalar(out=maskz[:, 0, :], in0=it_f[:],
                                scalar1=0.0, scalar2=None, op0=OP.not_equal)
        nc.vector.tensor_scalar(out=maskz[:, 1, :], in0=it_f[:],
                                scalar1=float(C - 1), scalar2=None,
                                op0=OP.not_equal)
        zero_h = sing.tile([128, 2, 128], bf, name="zeroh", tag="zeroh")
        nc.vector.memset(zero_h[:], 0.0)

        for _rep in range(n_rep):
            # ================= LSTM phases =================
            with ExitStack() as lctx:
                psp = lctx.enter_context(
                    tc.tile_pool(name="psp", bufs=3, space="PSUM"))
                ohp = lctx.enter_context(tc.tile_pool(name="ohp", bufs=3))
                sigp = lctx.enter_context(tc.tile_pool(name="sigp", bufs=3))
                tgp = lctx.enter_context(tc.tile_pool(name="tgp", bufs=3))
                t1p = lctx.enter_context(tc.tile_pool(name="t1p", bufs=3))
                tcp = lctx.enter_context(tc.tile_pool(name="tcp", bufs=3))
                cp = lctx.enter_context(tc.tile_pool(name="cp", bufs=4))
                whh = {(0, 0): wv("whh0_f"), (0, 1): wv("whh0_b"),
                       (1, 0): wv("whh1_f"), (1, 1): wv("whh1_b")}
                tabs = {(0, 0): wv("tab_lo_f"), (0, 1): wv("tab_hi_f"),
                        (1, 0): wv("tab_lo_b"), (1, 1): wv("tab_hi_b")}
                wih1 = {(0, 0): wv("wih1_ff"), (0, 1): wv("wih1_fb"),
                        (1, 0): wv("wih1_bf"), (1, 1): wv("wih1_bb")}

                for layer in range(2):
                    c_prev = []
                    for d in range(2):
                        c0 = cp.tile([128, 128], bf, name=f"c{d}", tag=f"c{d}")
                        nc.vector.memset(c0[:], 0.0)
                        c_prev.append(c0)
                    for tau in range(L):
                        if layer == 0:
                            oh_lo = ohp.tile([128, 2 * NCH], bf, name="ohlo",
                                             tag="ohlo")
                            oh_hi = ohp.tile([128, 2 * NCH], bf, name="ohhi",
                                             tag="ohhi")
                            for d in range(2):
                                nc.vector.tensor_scalar(
                                    out=fap(oh_lo[:], d * NCH,
                                            [[C, BL], [1, C]]),
                                    in0=seq_view(d, tau),
                                    scalar1=iota_f[:, 0:1], scalar2=None,
                                    op0=OP.is_equal)
                                nc.vector.tensor_scalar(
                                    out=fap(oh_hi[:], d * NCH,
                                            [[C, BL], [1, C]]),
                                    in0=seq_view(d, tau),
                                    scalar1=iota_f[:, 1:2], scalar2=None,
                                    op0=OP.is_equal)
                        # breadth-first emission: engines are strict FIFO, so
                        # stage-by-stage across both directions keeps the two
                        # recurrence chains pipelined.
                        g_ps = {}
                        for d in range(2):
                            g_ps[d] = psp.tile([128, 512], f32,
                                               name=f"g{d}", tag=f"g{d}")
                            if layer == 0:
                                nc.tensor.matmul(
                                    out=g_ps[d][:],
                                    lhsT=oh_lo[:, d * NCH:(d + 1) * NCH],
                                    rhs=tabs[(d, 0)],
                                    start=True, stop=False)
                                nc.tensor.matmul(
                                    out=g_ps[d][:],
                                    lhsT=oh_hi[:, d * NCH:(d + 1) * NCH],
                                    rhs=tabs[(d, 1)],
                                    start=False, stop=False)
                            else:
                                nc.tensor.matmul(out=g_ps[d][:],
                                                 lhsT=ones1[:],
                                                 rhs=b1row[:, d, :],
                                                 start=True, stop=False)
                                nc.tensor.matmul(out=g_ps[d][:],
                                                 lhsT=h_in(0, d, tau),
                                                 rhs=wih1[(d, 0)],
                                                 start=False, stop=False)
                                nc.tensor.matmul(out=g_ps[d][:],
                                                 lhsT=h_in(1, d, tau),
                                                 rhs=wih1[(d, 1)],
                                                 start=False, stop=False)
                        for d in range(2):
                            prev = (zero_h[:, d, :] if tau == 0
                                    else h_rw(layer, d, tau - 1))
                            nc.tensor.matmul(out=g_ps[d][:], lhsT=prev,
                                             rhs=whh[(layer, d)],
                                             start=False, stop=True)
                        sig = {}
                        for d in range(2):
                            sig[d] = sigp.tile([128, 512], bf,
                                               name=f"sig{d}", tag=f"sig{d}")
                            nc.scalar.activation(out=sig[d][:],
                                                 in_=g_ps[d][:],
                                                 func=AF.Sigmoid)
                        tg_t = {}
                        for d in range(2):
                            tg_t[d] = tgp.tile([128, 128], bf,
                                               name=f"tg{d}", tag=f"tg{d}")
                            nc.vector.tensor_scalar(
                                out=tg_t[d][:], in0=sig[d][:, 384:512],
                                scalar1=2.0, scalar2=1.0,
                                op0=OP.mult, op1=OP.subtract)
                        t1 = {}
                        for d in range(2):
                            t1[d] = t1p.tile([128, 128], bf,
                                             name=f"t1{d}", tag=f"t1{d}")
                            nc.vector.tensor_mul(t1[d][:], sig[d][:, 0:128],
                                                 tg_t[d][:])
                        c_new = {}
                        for d in range(2):
                            c_new[d] = cp.tile([128, 128], bf,
                                               name=f"c{d}", tag=f"c{d}")
                            nc.vector.tensor_mul(c_new[d][:],
                                                 sig[d][:, 128:256],
                                                 c_prev[d][:])
                        for d in range(2):
                            nc.vector.tensor_add(c_new[d][:], c_new[d][:],
                                                 t1[d][:])
                            if tau == W - 1:
                                nc.vector.tensor_mul(c_new[d][:], c_new[d][:],
                                                     maskz[:, d, :])
                        tc_t = {}
                        for d in range(2):
                            tc_t[d] = tcp.tile([128, 128], bf,
                                               name=f"tct{d}", tag=f"tct{d}")
                            nc.scalar.activation(out=tc_t[d][:],
                                                 in_=c_new[d][:],
                                                 func=AF.Tanh)
                        for d in range(2):
                            dst = h_rw(layer, d, tau)
                            src0 = fap(sig[d][:], 256, [[C, BL], [1, C]])
                            src1 = fap(tc_t[d][:], 0, [[C, BL], [1, C]])
                            nc.vector.tensor_mul(dst, src0, src1)
                        c_prev = [c_new[0], c_new[1]]

            # ================= emissions + CRF =================
            with ExitStack() as cctx:
                psp2 = cctx.enter_context(
                    tc.tile_pool(name="psp2", bufs=2, space="PSUM"))
                crf = cctx.enter_context(tc.tile_pool(name="crf", bufs=1))
                btag_sb = crf.tile([3, 1], f32, name="btag", tag="btag")
                nc.sync.dma_start(
                    out=btag_sb[:],
                    in_=fap(ins["smalls"], SM_BTAG, [[1, 1]],
                            part=[SM_COLS, 3]))
                em_all = crf.tile([32, TOK], f16, name="emall", tag="emall")
                nc.vector.memset(em_all[:], 0.0)
                em_T = crf.tile([128, NQ, 32], f16, name="emT", tag="emT")

                for k in range(TOK // 512):
                    em_ps = psp2.tile([3, 512], f32, name="em", tag="em")
                    nc.tensor.matmul(
                        out=em_ps[:], lhsT=wv("wtag_f", 3),
                        rhs=h_sb[(1, 0)][:, PAD + 512 * k:PAD + 512 * (k + 1)],
                        start=True, stop=False)
                    nc.tensor.matmul(
                        out=em_ps[:], lhsT=wv("wtag_b", 3),
                        rhs=h_sb[(1, 1)][:, PAD + 512 * k:PAD + 512 * (k + 1)],
                        start=False, stop=True)
                    nc.scalar.activation(
                        out=em_all[0:3, 512 * k:512 * (k + 1)], in_=em_ps[:],
                        func=AF.Identity, bias=btag_sb[:, 0:1])
                nc.sync.dma_start_transpose(out=em_T[:], in_=em_all[:])

                em_F = crf.tile([128, NQ, 3], f32, name="emF", tag="emF")
                nc.vector.tensor_copy(out=em_F[:], in_=em_T[:, :, 0:3])

                trans9 = crf.tile([128, 9], f32, name="trans9", tag="trans9")
                nc.sync.dma_start(
                    out=trans9[:],
                    in_=fap(ins["smalls"], SM_TRANS, [[1, 9]], part=[0, 128]))
                startr = crf.tile([128, 3], f32, name="startr", tag="startr")
                nc.sync.dma_start(
                    out=startr[:],
                    in_=fap(ins["smalls"], SM_STARTR, [[1, 3]], part=[0, 128]))
                i3_i = crf.tile([128, 3], i32, name="i3i", tag="i3i")
                nc.gpsimd.iota(i3_i[:], pattern=[[1, 3]], base=0,
                               channel_multiplier=0)
                i3 = crf.tile([128, 3], f32, name="i3", tag="i3")
                nc.vector.tensor_copy(out=i3[:], in_=i3_i[:])

                oh_cur = crf.tile([128, NQ, 3], f32, name="ohcur", tag="ohcur")
                oh_prev = crf.tile([128, NQ, 3], f32, name="ohprev",
                                   tag="ohprev")
                nc.vector.tensor_tensor(
                    out=oh_cur[:], in0=fap(tgw[:], 0, [[1, NQ], [0, 3]]),
                    in1=fap(i3[:], 0, [[0, NQ], [1, 3]]), op=OP.is_equal)
                nc.vector.tensor_tensor(
                    out=oh_prev[:], in0=fap(tgpw[:], 0, [[1, NQ], [0, 3]]),
                    in1=fap(i3[:], 0, [[0, NQ], [1, 3]]), op=OP.is_equal)

                # gold emission sum
                gtmp = crf.tile([128, NQ, 3], f32, name="gtmp", tag="gtmp")
                nc.vector.tensor_mul(gtmp[:], em_F[:], oh_cur[:])
                gold_r = crf.tile([128, 1], f32, name="goldr", tag="goldr")
                nc.vector.tensor_reduce(out=gold_r[:], in_=gtmp[:], axis=AX.XY,
                                        op=OP.add)
                # transition gold sum
                p2 = crf.tile([128, NQ, 3, 3], f32, name="p2", tag="p2")
                nc.vector.tensor_tensor(
                    out=p2[:], in0=fap(oh_prev[:], 0, [[3, NQ], [1, 3], [0, 3]]),
                    in1=fap(oh_cur[:], 0, [[3, NQ], [0, 3], [1, 3]]),
                    op=OP.mult)
                nc.vector.tensor_mul(p2[:], p2[:],
                                     fap(trans9[:], 0, [[0, NQ], [3, 3], [1, 3]]))
                trans_r = crf.tile([128, 1], f32, name="transr", tag="transr")
                nc.vector.tensor_reduce(out=trans_r[:], in_=p2[:], axis=AX.XYZ,
                                        op=OP.add)

                # transition matrices M_t[i,j] = trans[i,j] + em[t, j]
                M = crf.tile([128, NQ, 3, 3], f32, name="M", tag="M")
                nc.vector.tensor_tensor(
                    out=M[:], in0=fap(em_F[:], 0, [[3, NQ], [0, 3], [1, 3]]),
                    in1=fap(trans9[:], 0, [[0, NQ], [3, 3], [1, 3]]), op=OP.add)
                # slot t=0 of each seq -> A0 matrix: row0 = start + em[0], else -1e9
                for b in range(BL):
                    sl = M[32 * b:32 * b + 1, 0, :, :]
                    nc.vector.memset(sl, -1e9)
                    nc.vector.tensor_tensor(
                        out=M[32 * b:32 * b + 1, 0, 0, :],
                        in0=em_F[32 * b:32 * b + 1, 0, :],
                        in1=startr[32 * b:32 * b + 1, :], op=OP.add)

                # in-partition tree levels: 64 -> 1 matrices per partition.
                # ISA allows max 3 free AP dims, so the (pair,i,j,k) expand is
                # emitted as 3 ops (one per output row i).
                def tree_levels(cur, nmat, pdim):
                    while nmat > 1:
                        n2 = nmat // 2
                        X = crf.tile([pdim, max(n2, 1), 3, 3, 3], f32,
                                     name=f"X{pdim}_{n2}", tag=f"X{pdim}_{n2}")
                        for i in range(3):
                            # X[pair, i, j, k] = A[pair, i, k] + B[pair, k, j]
                            out_i = fap(X[:], i * 9, [[27, n2], [3, 3], [1, 3]])
                            A_i = fap(cur[:], i * 3, [[18, n2], [0, 3], [1, 3]])
                            B_ = fap(cur[:], 9, [[18, n2], [1, 3], [3, 3]])
                            nc.vector.tensor_tensor(out=out_i, in0=A_i, in1=B_,
                                                    op=OP.add)
                        Xv = fap(X[:], 0, [[27, n2], [3, 9], [1, 3]])
                        mx = crf.tile([pdim, max(n2, 1), 3, 3], f32,
                                      name=f"mx{pdim}_{n2}", tag=f"mx{pdim}_{n2}")
                        nc.vector.tensor_reduce(out=mx[:], in_=Xv, axis=AX.X,
                                                op=OP.max)
                        nc.vector.tensor_tensor(
                            out=Xv, in0=Xv,
                            in1=fap(mx[:], 0, [[9, n2], [1, 9], [0, 3]]),
                            op=OP.subtract)
                        Xf = fap(X[:], 0, [[1, n2 * 27]])
                        nc.scalar.activation(out=Xf, in_=Xf, func=AF.Exp)
                        sm = crf.tile([pdim, max(n2, 1), 3, 3], f32,
                                      name=f"sm{pdim}_{n2}", tag=f"sm{pdim}_{n2}")
                        nc.vector.tensor_reduce(out=sm[:], in_=Xv, axis=AX.X,
                                                op=OP.add)
                        smf = fap(sm[:], 0, [[1, n2 * 9]])
                        nc.scalar.activation(out=smf, in_=smf, func=AF.Ln)
                        nxt = crf.tile([pdim, max(n2, 1), 3, 3], f32,
                                       name=f"nx{pdim}_{n2}", tag=f"nx{pdim}_{n2}")
                        nc.vector.tensor_tensor(out=nxt[:], in0=sm[:], in1=mx[:],
                                                op=OP.add)
                        cur, nmat = nxt, n2
                    return cur

                pr128 = tree_levels(M, NQ, 128)  # [128, 1, 3, 3]
                # compact across partitions via DRAM bounce
                scr_d = dram.tile([128, 9], f32, name="scrd", tag="scrd")
                nc.sync.dma_start(out=scr_d[:], in_=pr128[:])
                cmp = crf.tile([4, 32, 3, 3], f32, name="cmp", tag="cmp")
                nc.sync.dma_start(
                    out=cmp[:], in_=fap(scr_d[:], 0, [[9, 32], [3, 3], [1, 3]],
                                        part=[32 * 9, 4]))
                prfin = tree_levels(cmp, 32, 4)  # [4, 1, 3, 3]

                end3 = crf.tile([4, 3], f32, name="end3", tag="end3")
                oh0 = crf.tile([4, 3], f32, name="oh0", tag="oh0")
                ohl = crf.tile([4, 3], f32, name="ohl", tag="ohl")
                st3 = crf.tile([4, 3], f32, name="st3", tag="st3")
                for t_, off_ in ((end3, SM_END3), (oh0, SM_OH0),
                                 (ohl, SM_OHL), (st3, SM_START3)):
                    nc.sync.dma_start(
                        out=t_[:],
                        in_=fap(ins["smalls"], off_, [[1, 3]],
                                part=[SM_COLS, 4]))

                z2 = crf.tile([4, 3, 3], f32, name="z2", tag="z2")
                nc.vector.tensor_tensor(
                    out=z2[:], in0=fap(prfin[:], 0, [[3, 3], [1, 3]]),
                    in1=fap(end3[:], 0, [[0, 3], [1, 3]]), op=OP.add)
                mx4 = crf.tile([4, 1], f32, name="mx4", tag="mx4")
                nc.vector.tensor_reduce(out=mx4[:], in_=z2[:], axis=AX.XY,
                                        op=OP.max)
                nc.vector.tensor_tensor(
                    out=z2[:], in0=z2[:],
                    in1=fap(mx4[:], 0, [[0, 3], [0, 3]]), op=OP.subtract)
                nc.scalar.activation(out=z2[:], in_=z2[:], func=AF.Exp)
                s4 = crf.tile([4, 1], f32, name="s4", tag="s4")
                nc.vector.tensor_reduce(out=s4[:], in_=z2[:], axis=AX.XY,
                                        op=OP.add)
                nc.scalar.activation(out=s4[:], in_=s4[:], func=AF.Ln)
                den4 = crf.tile([4, 1], f32, name="den4", tag="den4")
                nc.vector.tensor_add(den4[:], s4[:], mx4[:])

                stmp = crf.tile([4, 3], f32, name="stmp", tag="stmp")
                nc.vector.tensor_mul(stmp[:], oh0[:], st3[:])
                sstart = crf.tile([4, 1], f32, name="sstart", tag="sstart")
                nc.vector.tensor_reduce(out=sstart[:], in_=stmp[:], axis=AX.X,
                                        op=OP.add)
                nc.vector.tensor_mul(stmp[:], ohl[:], end3[:])
                send = crf.tile([4, 1], f32, name="send", tag="send")
                nc.vector.tensor_reduce(out=send[:], in_=stmp[:], axis=AX.X,
                                        op=OP.add)
                se = crf.tile([4, 1], f32, name="se", tag="se")
                nc.vector.tensor_add(se[:], sstart[:], send[:])

                out_sb = crf.tile([128, 4], f32, name="outsb", tag="outsb")
                nc.vector.memset(out_sb[:], 0.0)
                nc.vector.tensor_copy(out=out_sb[:, 0:1], in_=gold_r[:])
                nc.vector.tensor_copy(out=out_sb[:, 1:2], in_=trans_r[:])
                nc.vector.tensor_copy(out=out_sb[0:4, 2:3], in_=den4[:])
                nc.vector.tensor_copy(out=out_sb[0:4, 3:4], in_=se[:])
                nc.sync.dma_start(out=outs["outp"], in_=out_sb[:])


def combine_out(outp):
    """outp: [128, 4] fp32 per core -> partial loss (den - num)."""
    num = outp[:, 0].sum() + outp[:, 1].sum() + outp[0:4, 3].sum()
    den = outp[0:4, 2].sum()
    return den - num


_CACHE = {}


def _get_program():
    if "nc" in _CACHE:
        return _CACHE["nc"], _CACHE["ins"], _CACHE["outs"]
    import concourse.bacc as bacc
    import concourse.tile as tile
    from concourse import mybir

    nc = bacc.Bacc("TRN2", target_bir_lowering=False, debug=False,
                   num_devices=NCORES)
    dtmap = {"bf16": mybir.dt.bfloat16, "f32": mybir.dt.float32,
             "u8": mybir.dt.uint8}
    ins = {}
    for nm, shp, dt_ in INPUT_SPECS:
        ins[nm] = nc.dram_tensor(nm, list(shp), dtmap[dt_],
                                 kind="ExternalInput").ap()
    outs = {
        "outp": nc.dram_tensor("outp", [128, 4], mybir.dt.float32,
                               kind="ExternalOutput").ap(),
    }
    with tile.TileContext(nc) as tc:
        build(tc, ins, outs)
    nc.compile()
    _CACHE.update(nc=nc, ins=ins, outs=outs)
    return nc, ins, outs


def kernel(**inputs):
    from concourse.bass_utils import run_bass_kernel_spmd

    per_core = host_prep(inputs)
    nc, ins, outs = _get_program()
    in_maps = []
    for k in range(NCORES):
        m = {nm: np.ascontiguousarray(per_core[k][nm])
             for nm, _, _ in INPUT_SPECS}
        in_maps.append(m)
    res = run_bass_kernel_spmd(
        nc, in_maps, core_ids=list(range(NCORES)),
        trace=bool(int(os.environ.get("BASS_PROFILE", "0"))))
    total = 0.0
    for k in range(NCORES):
        total += combine_out(res.results[k]["outp"])
    if res.exec_time_ns is not None:
        kernel.last_exec_ns = res.exec_time_ns
    return np.float32(total)


kernel.last_exec_ns = None
